# revision 42
# baseline (speedup 1.0000x reference)
"""Trainium2 Bass kernel for SKalmanNet GSS (dense GEMV chain, batch=1).

Strategy (8 NeuronCores, 2 branches x 4-way tensor parallel):
  - Cores 0-3 compute Pk (l1/gru1/l2), cores 4-7 compute Sk (replica
    groups [[0,1,2,3],[4,5,6,7]]), one SPMD program.
  - Weight dtypes: l1 / gru Wih / gru Whh in fp8-e3m4 (per-tensor
    power-of-2 scale folded into the host-prepped activations), l2_W1 /
    l2_W2 in fp16 (fp8 there breaks the 2e-2 budget and their DMA is off
    the critical path).  Measured end-to-end rel err ~1.1e-2 (tol 2e-2).
  - A tiny dummy ReduceScatter triggered at ~t=0 absorbs the runtime's
    global entry barrier + first-collective stream warmup under the
    weight DMA; launch skew across cores is up to ~50us.
  - PE program order matches DMA arrival order (l1, gih, then gh during
    the RS1 wait) so RS1's trigger — the group-critical event — is never
    queued behind whh's later arrival.
  - Collective layout (2 collectives instead of 3):
      l1   row-sharded    -> AllGather(l1_out)        [collective 1]
      gru  row-sharded    -> h' shard stays local
      l2_W1 COLUMN-sharded by h-dims -> partial [4096] -> ReduceScatter
                                                         [collective 2]
      l2_W2 COLUMN-sharded by the RS output chunk -> partial [1024] out
      host sums the 4 partial outputs per branch (+ b2).
  - All weights are SBUF-resident (no ring reuse): DMA streams ~18.8MB
    per core with zero stalls.  wl1 + collective staging ride the ACT
    HWDGE ring so the l1->AllGather trigger path never queues behind
    the bulk weight stream (SP ring).
  - GEMVs run 3 PE column-group "chains" (tile_position col offsets
    0/32/64): 3 concurrent matmul streams (~2.4x PE throughput), one
    m-tile per chain, K-accumulated in PSUM, no cross-chain merge.
  - Elementwise stages avoid [1,N] single-lane layouts: psum quadrants
    are copied in one [65,512] op, gate math runs partition-major
    [128,4] after tiny K=1 matmul transposes.
"""

import os

import numpy as np

X = 32
Y = 32
H1 = 5120
HID = 2048
H2 = 4096
OUT = 1024
IN = 1120
INP = 1152          # 9*128, slot 1120 = bias

NCORES = 8
TP = 4

M_L1 = H1 // TP     # 1280
M_G = 3 * (HID // TP)   # 1536 (r|z|n x 512)
HSH = HID // TP     # 512
Y3C = H2 // TP      # 1024 (ReduceScatter output chunk)

K_L1 = INP // 128   # 9
K_IH = H1 // 128    # 40
K_HH = HID // 128   # 16
K_W1 = HSH // 128   # 4
K_W2 = Y3C // 128   # 8

PL1 = 1536          # padded AllGather width (48*32, xbar-transposable)

# partition-major consts [128, CPM_TOT] f32: each [512] vec as [128,4]
CPM_BRZ_R = 0       # bih+bhh, r gate
CPM_BRZ_Z = 4
CPM_BHHN = 8
CPM_BIHN = 12
CPM_HSH = 16
CPM_B1 = 20         # l2_b1 RS chunk as [128, 8]
CPM_TOT = 28

FP8 = os.environ.get("KERNEL_FP8", "1") == "1"
FMAX = 12.0         # target max for e3m4 scaling (format max 15.5)

_CACHE = {}


def _build_nc(s_ih, s_w1, s_w2):
    import concourse.bass as bass  # noqa: F401
    import concourse.mybir as mybir
    import concourse.tile as tile
    from concourse import bacc

    f32 = mybir.dt.float32
    f16 = mybir.dt.float16
    wq = mybir.dt.float8e3 if FP8 else mybir.dt.float16

    nc = bacc.Bacc("TRN2", target_bir_lowering=False, debug=False,
                   num_devices=NCORES)

    idm_d = nc.dram_tensor("idm", [12, 12], f16, kind="ExternalInput")
    x0_d = nc.dram_tensor("x0", [128, K_L1], f16, kind="ExternalInput")
    hx_d = nc.dram_tensor("hx", [128, K_HH], f16, kind="ExternalInput")
    cpm_d = nc.dram_tensor("cpm", [128, CPM_TOT], f32, kind="ExternalInput")
    wl1_d = nc.dram_tensor("wl1", [3, 128, 3 * M_L1], wq, kind="ExternalInput")
    wih_d = nc.dram_tensor("wih", [4, 128, 10 * M_G], wq,
                           kind="ExternalInput")
    whh_d = nc.dram_tensor("whh", [1, 128, K_HH * M_G], wq,
                           kind="ExternalInput")
    w1_d = nc.dram_tensor("w1", [2, 128, K_W1 * H2 // 2], wq,
                          kind="ExternalInput")
    w2_d = nc.dram_tensor("w2", [1, 128, K_W2 * OUT], f16,
                          kind="ExternalInput")
    out_d = nc.dram_tensor("out", [4, 256], f32, kind="ExternalOutput")

    AF = mybir.ActivationFunctionType
    ALU = mybir.AluOpType
    groups = [[0, 1, 2, 3], [4, 5, 6, 7]]

    with tile.TileContext(nc) as tc, \
         tc.tile_pool(name="act", bufs=1) as apool, \
         tc.tile_pool(name="ps", bufs=8, space="PSUM") as ppool, \
         tc.tile_pool(name="dram", bufs=1, space="DRAM") as dpool:
        wpool = apool  # one SBUF pool: fewer TileContext exit barriers

        # ---- prelude-AllGather warm-up: registering these replica groups
        # makes the framework insert a tiny AllGather right after the
        # gpsimd preamble (doorbell at ~7us).  It absorbs the runtime's
        # global 8-core entry barrier (fires at the first collective,
        # ~15-20us AFTER the last core arrives — launch skew is up to
        # ~50us) plus the first-collective stream warmup, overlapped with
        # the weight DMA stream.  Unlike a user dummy collective_compute,
        # it completes ~8us earlier on the stream, so RS1 isn't pushed out.
        # Registered directly (not via bir_kernel_barrier_wait) because the
        # Tile scheduler's sim can't see the finalize-time AG increment and
        # would deadlock on the wait_ge; the AG instruction itself already
        # blocks gpsimd until the entry sync completes, which is all the
        # ordering RS1 needs.
        assert nc._bir_kernel_barrier_sem is not None
        nc._bir_kernel_barrier_sem_replica_groups.extend(
            set(g) for g in groups)

        # ---- input / consts DMAs (ACT HWDGE ring: lower fixed cost than
        # SWDGE and keeps GpSimd free to run only the collective stream)
        cpm = apool.tile([128, CPM_TOT], f32, tag="cpm", name="cpm_sb")
        nc.scalar.dma_start(cpm, cpm_d.ap())
        x0 = apool.tile([128, K_L1], f16, tag="x0", name="x0")
        nc.scalar.dma_start(x0, x0_d.ap())
        hx = apool.tile([128, K_HH], f16, tag="hx", name="hx")
        nc.scalar.dma_start(hx, hx_d.ap())
        idm = apool.tile([12, 12], f16, tag="idm", name="idm")
        nc.scalar.dma_start(idm, idm_d.ap())

        # ---- resident weight buffers
        wl1 = wpool.tile([128, K_L1 * M_L1], wq, tag="wl1", name="wl1_sb")
        wih = wpool.tile([128, K_IH * M_G], wq, tag="wih", name="wih_sb")
        whh = wpool.tile([128, K_HH * M_G], wq, tag="whh", name="whh_sb")
        w1 = wpool.tile([128, K_W1 * H2], wq, tag="w1", name="w1_sb")
        w2 = wpool.tile([128, K_W2 * OUT], f16, tag="w2", name="w2_sb")
        # wl1 heads the SP-ring queue so l1 can start before the bulk
        # stream saturates HBM (both HWDGE rings share the SDMA engines,
        # so a separate ring gives no priority).
        for p in range(3):
            nc.sync.dma_start(wl1[:, p * 3 * M_L1:(p + 1) * 3 * M_L1],
                              wl1_d.ap()[p])
        # wih directly after wl1: the l1->gih->RS1-trigger chain is the
        # group-critical path (RS1 completes only after the LAST core's
        # trigger); whh is only needed for gh, which hides under RS1.
        for p in range(4):
            nc.sync.dma_start(wih[:, p * 10 * M_G:(p + 1) * 10 * M_G],
                              wih_d.ap()[p])
        nc.sync.dma_start(whh[:, :], whh_d.ap()[0])
        for p in range(2):
            nc.sync.dma_start(w1[:, p * 2 * H2:(p + 1) * 2 * H2],
                              w1_d.ap()[p])
        nc.sync.dma_start(w2[:, :], w2_d.ap()[0])

        # ---- ACT LUT warmup
        warm = apool.tile([1, 32], f32, tag="warm", name="warm")
        nc.vector.memset(warm, 0.0)
        nc.scalar.activation(warm, warm, AF.Sigmoid)
        nc.scalar.activation(warm, warm, AF.Tanh)
        nc.scalar.activation(warm, warm, AF.Relu)

        ones = apool.tile([128, 1], f16, tag="ones", name="ones")
        nc.vector.memset(ones, 1.0)

        # ---- PSUM: 8 banks, allocated up-front.  The [65,512]-style
        # batched psum->SBUF copies read quadrant-gap rows, so zero those
        # regions early (off the critical path).
        l1p = ppool.tile([128, 512], f32, tag="l1p", bufs=1, name="l1p")
        ghp = ppool.tile([128, 512], f32, tag="ghp", bufs=1, name="ghp")
        gip = ppool.tile([128, 512], f32, tag="gip", bufs=1, name="gip")
        w1ps = [ppool.tile([128, 512], f32, tag=f"w1p{b}", bufs=1,
                           name=f"w1p{b}") for b in range(2)]
        w2p = l1p  # l1p's bank is free after y1; reuse for the W2 output
        smalls = ppool.tile([128, 42], f32, tag="smalls", bufs=1,
                            name="smalls")
        ghtp = smalls[:, 0:12]
        gtp = smalls[:, 12:24]
        x3ps = smalls[:, 24:32]
        xlps = smalls[:, 32:42]
        nc.vector.memset(l1p[0:65, :], 0.0)
        nc.vector.memset(gip[0:97, :], 0.0)
        nc.vector.memset(w1ps[0][0:97, :], 0.0)
        nc.vector.memset(w1ps[1][0:97, :], 0.0)

        def gemv(x_sb, w_sb, K, M, acc, xmap=None, tw=512, pb=3):
            """acc[j//pb][32*(j%pb), :mw] = W @ x for m-tile j (tw wide).
            pb concurrent column-group chains, K-accumulated in psum.
            k-major: one LDW per chain per k, all m-tiles streamed."""
            nm = (M + tw - 1) // tw
            mts = [(i * tw, min(tw, M - i * tw)) for i in range(nm)]

            def mm(j, k):
                kk = xmap(k) if xmap else k
                m0, mw = mts[j]
                c = 32 * (j % pb)
                nc.tensor.matmul(
                    acc[j // pb][c:c + 1, :mw],
                    x_sb[:, kk:kk + 1],
                    w_sb[:, k * M + m0: k * M + m0 + mw],
                    start=(k == 0), stop=(k == K - 1),
                    tile_position=(0, c),
                )

            # k-major for the bulk; the tail chunks run m-tile-major so
            # early m-tiles retire first and their psum->SBUF copies
            # overlap the remaining chains.  Small-K gemvs (W1) go fully
            # m-tile-major so the first bank's staging overlaps the rest.
            tail = K if K <= 4 else (min(6, K) if K > 6 else 0)
            for k in range(K - tail):
                for j in range(nm):
                    mm(j, k)
            for j in range(nm):
                for k in range(K - tail, K):
                    mm(j, k)
            return mts

        def to_part(src, col0, cols, ps, pcol0):
            """src[0, col0+128*c : col0+128*(c+1)] (fp16 SBUF, partition 0)
            -> ps[:, pcol0+c] for c in range(cols), via K=1 matmuls."""
            for c in range(cols):
                s0 = col0 + c * 128
                nc.tensor.matmul(ps[:, pcol0 + c:pcol0 + c + 1],
                                 src[0:1, s0:s0 + 128],
                                 ones[0:1, 0:1],
                                 start=True, stop=True)

        # ---- l1: relu(W@x)*s_ih, row-sharded (bias folded in weights)
        gemv(x0, wl1, K_L1, M_L1, [l1p])
        # ---- l1 output stays local: one [65,512] relu keeps all lanes
        # busy (vs 3 serial single-lane [1,mw] ops), then on-PE transpose
        # to x1loc [128,10] for the column-sharded gi.
        ytmp = apool.tile([65, 512], f16, tag="ytmp", name="ytmp")
        nc.scalar.activation(ytmp, l1p[0:65, :], AF.Relu, scale=float(s_ih))
        for c in range(10):
            r, q = 32 * (c // 4), (c % 4) * 128
            nc.tensor.matmul(xlps[:, c:c + 1],
                             ytmp[r:r + 1, q:q + 128],
                             ones[r:r + 1, 0:1],
                             start=True, stop=True)
        x1loc = apool.tile([128, 10], f16, tag="x1loc", name="x1loc")
        nc.scalar.copy(x1loc, xlps)

        # gih BEFORE the gh gemv in PE order: wih is streamed right after
        # wl1 (whh after), and the PE executes its queue in order — with
        # gh first, gih's tail (and so RS1's trigger, the group-critical
        # event) would be queued behind whh's later DMA arrival.
        gacc = [gip, w1ps[0], w1ps[1]]
        gemv(x1loc, wih, 10, 3 * HID, gacc, tw=512, pb=4)
        rsin1 = dpool.tile([12, 512], f16, tag="rsin1", name="rsin1")
        rs1out = dpool.tile([12, 128], f16, tag="rs1out", name="rs1out")
        for b in range(3):
            yg = apool.tile([97, 512], f16, tag=f"yg{b}", name=f"yg{b}")
            nc.scalar.copy(yg, gacc[b][0:97, :])
            nc.scalar.dma_start(rsin1[4 * b:4 * b + 4, :], yg[0:97:32, :])
        nc.gpsimd.collective_compute(
            "ReduceScatter", ALU.add, replica_groups=groups,
            ins=[rsin1.opt()], outs=[rs1out.opt()])

        # ---- gru: gh = Whh @ (hn*s_hh); runs during the RS1 wait
        gemv(hx, whh, K_HH, M_G, [ghp], tw=384, pb=4)
        ghs = apool.tile([1, M_G], f16, tag="ghs", name="ghs")
        for g in range(4):
            nc.scalar.copy(ghs[:, 384 * g:384 * (g + 1)],
                           ghp[32 * g:32 * g + 1, :384])
        to_part(ghs, 0, 12, ghtp, 0)
        # ghb = gh + [brz_r | brz_z | bhhn]  (partition-major [128,12])
        ghb = apool.tile([128, 12], f32, tag="ghb", name="ghb")
        nc.vector.tensor_add(ghb, ghtp, cpm[:, 0:12])

        # RS1 result lands as [12,128] in SBUF; one identity matmul
        # transposes it to partition-major [128,12] (vs 12 serial K=1
        # matmuls) — this sits on the group-critical post-RS1 path.
        gis = apool.tile([12, 128], f16, tag="gis", name="gis")
        nc.scalar.dma_start(gis, rs1out)
        nc.tensor.matmul(gtp, gis, idm, start=True, stop=True)

        # ---- gru cell elementwise, partition-major [128, 4] per gate
        rz = apool.tile([128, 8], f32, tag="rz", name="rz")
        nc.vector.tensor_add(rz, gtp[:, 0:8], ghb[:, 0:8])
        nc.scalar.activation(rz, rz, AF.Sigmoid)
        tn = apool.tile([128, 4], f32, tag="tn", name="tn")
        nc.vector.tensor_mul(tn, rz[:, 0:4], ghb[:, 8:12])      # r*(ghn+bhhn)
        tn2 = apool.tile([128, 4], f32, tag="tn2", name="tn2")
        nc.vector.tensor_add(tn2, gtp[:, 8:12], cpm[:, CPM_BIHN:CPM_BIHN + 4])
        nc.vector.tensor_add(tn, tn, tn2)
        nc.scalar.activation(tn, tn, AF.Tanh)                   # n
        # s_w1 (the fp8 scale of l2_W1) is folded into h' here for free:
        # cpm's HSH column holds h*s_w1 (host-prepped), so
        # td = s*h - s*n, x2 = s*n + z*(s*h - s*n) = s_w1 * h'.
        td = apool.tile([128, 4], f32, tag="td", name="td")
        nc.vector.scalar_tensor_tensor(td, tn, -float(s_w1),
                                       cpm[:, CPM_HSH:CPM_HSH + 4],
                                       ALU.mult, ALU.add)       # s*(h-n)
        nc.vector.tensor_mul(td, rz[:, 4:8], td)                # z*s*(h-n)
        x2 = apool.tile([128, 4], f16, tag="x2", name="x2")
        nc.vector.scalar_tensor_tensor(x2, tn, float(s_w1), td,
                                       ALU.mult, ALU.add)       # s_w1*h'

        # ---- l2_W1 column-sharded: partial[4096] = W1[:, shard] @ h'
        gemv(x2, w1, K_W1, H2, w1ps, pb=4)
        rsin = dpool.tile([8, 512], f16, tag="rsin", name="rsin")
        rsout = dpool.tile([8, 128], f16, tag="rsout", name="rsout")
        for b in range(2):
            yb = apool.tile([97, 512], f16, tag=f"yb{b}", name=f"yb{b}")
            nc.scalar.copy(yb, w1ps[b][0:97, :])
            nc.scalar.dma_start(rsin[4 * b:4 * b + 4, :], yb[0:97:32, :])

        # keep-warm junk matmuls: span the RS2 wait (staging + ~6.5us RS +
        # y3p DMA ~= 12us) so the PE HAM throttle doesn't re-engage before
        # the y3/W2 tail.  Slightly undersized: overshoot delays the y3
        # transpose directly, undershoot only risks a 2x-slower W2.
        for i in range(30):
            nc.tensor.matmul(ghp[0:1, :384], x2[:, 0:1], w2[:, 0:384],
                             start=True, stop=True, tile_position=(0, 0))
        nc.gpsimd.collective_compute(
            "ReduceScatter", ALU.add, replica_groups=groups,
            ins=[rsin.opt()], outs=[rsout.opt()])
        y3p = apool.tile([8, 128], f16, tag="y3p", name="y3p")
        nc.scalar.dma_start(y3p, rsout)

        # transpose to [128, 8] with one identity matmul, then bias+relu
        nc.tensor.matmul(x3ps, y3p, idm[0:8, 0:8], start=True, stop=True)
        x3t = apool.tile([128, 8], f32, tag="x3t", name="x3t")
        nc.vector.tensor_add(x3t, x3ps, cpm[:, CPM_B1:CPM_B1 + 8])
        x3 = apool.tile([128, 8], f16, tag="x3", name="x3")
        # relu(x*s) = s*relu(x) for s>0: folds the fp8 scale of l2_W2 in.
        nc.scalar.activation(x3, x3t, AF.Relu, scale=float(s_w2))

        # ---- l2_W2 column-sharded: partial [1024] out.  tw=256/pb=4 runs
        # 4 concurrent PE column-group chains (2x the 2-chain tw=512 rate).
        gemv(x3, w2, K_W2, OUT, [w2p], tw=256, pb=4)
        yo = apool.tile([97, 256], f32, tag="yo", name="yo")
        nc.scalar.copy(yo, w2p[0:97, :256])
        nc.scalar.dma_start(out_d.ap(), yo[0:97:32, :])

    nc.finalize()
    return nc


def _pow2_scale(*arrs):
    m = max(float(np.abs(a).max()) for a in arrs)
    return float(2.0 ** np.ceil(np.log2(max(m, 1e-30) / FMAX)))


def _qpack(wt, K, M, npieces, npw):
    """[K*128, M] input-major transposed weight -> [npieces, 128, K*M/np]
    chunk-major packed (element [p, k*M+m] = wt[k*128+p, m])."""
    v = (wt.reshape(K, 128, M).transpose(1, 0, 2)
         .reshape(128, npieces, K * M // npieces).transpose(1, 0, 2))
    return np.ascontiguousarray(v).astype(npw)


def _pm(vec):
    """[n*128] -> [128, n] partition-major (element u -> [u%128, u//128])."""
    return np.ascontiguousarray(np.asarray(vec, np.float32)
                                .reshape(-1, 128).T)


def _prep_core(r, xvec, hn, l1W, l1b, Wih, Whh, bih, bhh, W1, b1, W2, b2,
               s_l1, s_ih, s_hh, s_w1, s_w2, npw):
    f32 = np.float32
    f16 = np.float16

    rs = slice(r * M_L1, (r + 1) * M_L1)
    wt = np.zeros((INP, M_L1), f32)
    wt[:IN] = l1W[rs].T
    wt[IN] = l1b[rs]
    wl1 = _qpack(wt / s_l1, K_L1, M_L1, 3, npw)

    gsl = [slice(g * HID + r * HSH, g * HID + (r + 1) * HSH) for g in range(3)]
    gidx = np.concatenate([np.arange(s.start, s.stop) for s in gsl])
    # full rank-major gate permutation (rank r' block = r|z|n of shard r')
    pfull = np.concatenate([
        np.arange(g * HID + rr * HSH, g * HID + (rr + 1) * HSH)
        for rr in range(TP) for g in range(3)])
    wih = _qpack(np.ascontiguousarray(
        Wih[pfull][:, r * M_L1:(r + 1) * M_L1].T) / s_ih, 10, 3 * HID,
        4, npw)
    whh = _qpack(np.ascontiguousarray(Whh[gidx].T) / s_hh, K_HH, M_G, 1, npw)

    w1 = _qpack(np.ascontiguousarray(W1[:, r * HSH:(r + 1) * HSH].T) / s_w1,
                K_W1, H2, 2, npw)
    w2 = _qpack(np.ascontiguousarray(W2[:, r * Y3C:(r + 1) * Y3C].T) / s_w2,
                K_W2, OUT, 1, f16)

    bsum = bih + bhh
    cpm = np.concatenate([
        _pm(bsum[gsl[0]]), _pm(bsum[gsl[1]]),      # brz_r, brz_z
        _pm(bhh[gsl[2]]), _pm(bih[gsl[2]]),        # bhhn, bihn
        _pm(hn[r * HSH:(r + 1) * HSH]) * s_w1,     # h shard * s_w1
        _pm(b1[r * Y3C:(r + 1) * Y3C]),            # RS chunk bias
    ], axis=1).astype(f32)
    assert cpm.shape == (128, CPM_TOT)

    x = np.zeros(INP, f32)
    x[:IN] = xvec
    x[IN] = 1.0
    x0 = np.ascontiguousarray((x * s_l1).reshape(K_L1, 128).T).astype(f16)
    hx = np.ascontiguousarray((hn * s_hh).reshape(K_HH, 128).T).astype(f16)

    return {
        "x0": x0, "hx": hx, "cpm": cpm, "idm": np.eye(12, dtype=f16),
        "wl1": wl1, "wih": wih, "whh": whh, "w1": w1, "w2": w2,
    }


LAST_RESULT = None


def kernel(state_inno, observation_inno, diff_state, diff_obs,
           linearization_error, Jacobian,
           l1_W, l1_b, gru1_Wih, gru1_Whh, gru1_bih, gru1_bhh,
           l2_W1, l2_b1, l2_W2, l2_b2,
           l3_W, l3_b, gru2_Wih, gru2_Whh, gru2_bih, gru2_bhh,
           l4_W1, l4_b1, l4_W2, l4_b2, hn1, hn2):
    global LAST_RESULT
    from concourse.bass_utils import run_bass_kernel_spmd
    import concourse.mybir as mybir

    npw = mybir.dt.np(mybir.dt.float8e3) if FP8 else np.float16

    a = lambda v: np.asarray(v, dtype=np.float32)
    input1 = np.concatenate([a(state_inno), a(diff_state),
                             a(linearization_error), a(Jacobian)]).reshape(-1)
    input2 = np.concatenate([a(observation_inno), a(diff_obs),
                             a(linearization_error), a(Jacobian)]).reshape(-1)

    branches = [
        (input1, a(hn1).reshape(-1), a(l1_W), a(l1_b).reshape(-1),
         a(gru1_Wih), a(gru1_Whh), a(gru1_bih).reshape(-1),
         a(gru1_bhh).reshape(-1), a(l2_W1), a(l2_b1).reshape(-1),
         a(l2_W2), a(l2_b2).reshape(-1)),
        (input2, a(hn2).reshape(-1), a(l3_W), a(l3_b).reshape(-1),
         a(gru2_Wih), a(gru2_Whh), a(gru2_bih).reshape(-1),
         a(gru2_bhh).reshape(-1), a(l4_W1), a(l4_b1).reshape(-1),
         a(l4_W2), a(l4_b2).reshape(-1)),
    ]

    if FP8:
        s_l1 = _pow2_scale(
            np.concatenate([branches[0][2].ravel(), branches[0][3]]),
            np.concatenate([branches[1][2].ravel(), branches[1][3]]))
        s_ih = _pow2_scale(branches[0][4], branches[1][4])
        s_hh = _pow2_scale(branches[0][5], branches[1][5])
        s_w1 = _pow2_scale(branches[0][8], branches[1][8])
    else:
        s_l1 = s_ih = s_hh = s_w1 = 1.0
    # l2_W2 stays f16 (fp8 on BOTH l2 weights pushed rel err past the
    # 2e-2 budget; W2's quant error hits the output unaveraged).  W1 in
    # fp8 halves the critical-path W1 gemv's xbus time.
    s_w2 = 1.0

    if "nc" not in _CACHE:
        _CACHE["nc"] = (_build_nc(s_ih, s_w1, s_w2),
                        s_l1, s_ih, s_hh, s_w1, s_w2)
    nc, s_l1, s_ih, s_hh, s_w1, s_w2 = _CACHE["nc"]

    in_maps = [_prep_core(c % TP, *branches[c // TP],
                          s_l1, s_ih, s_hh, s_w1, s_w2, npw) for c in range(NCORES)]

    kwargs = {}
    if os.environ.get("KERNEL_TRACE"):
        cores = os.environ.get("KERNEL_TRACE_CORES", "0")
        kwargs.update(trace=True,
                      trace_cores=[int(c) for c in cores.split(",")])

    res = run_bass_kernel_spmd(nc, in_maps, core_ids=list(range(NCORES)),
                               **kwargs)
    LAST_RESULT = res
    outs = [res.results[c]["out"].reshape(-1) for c in range(NCORES)]
    b2P = branches[0][11]
    b2S = branches[1][11]
    Pk = (sum(outs[:TP]) + b2P).reshape(X, X).astype(np.float32)
    Sk = (sum(outs[TP:]) + b2S).reshape(Y, Y).astype(np.float32)
    return Pk, Sk



# revision 43
# speedup vs baseline: 1.6533x; 1.6533x over previous
"""Trainium2 Bass kernel for SKalmanNet GSS (dense GEMV chain, batch=1).

Strategy (8 NeuronCores, 2 branches x 4-way tensor parallel):
  - Cores 0-3 compute Pk (l1/gru1/l2), cores 4-7 compute Sk (replica
    groups [[0,1,2,3],[4,5,6,7]]), one SPMD program.
  - Weight dtypes: l1 / gru Wih / gru Whh / l2_W1 in fp8-e3m4
    (per-tensor power-of-2 scales folded into host-prepped activations
    or the gru output), l2_W2 in fp16 (fp8 on BOTH l2 weights broke the
    2e-2 budget).  Measured end-to-end rel err 1.823e-2 (tol 2e-2,
    deterministic — seeded inputs).
  - Collective layout (2 data collectives):
      l1    row-sharded    -> local x1 shard, no comm
      gru   Wih COLUMN-sharded by x1 shard -> partial gi [6144]
            -> ReduceScatter -> own gate shard [1536]   [collective 1]
      l2_W1 COLUMN-sharded by h-dims -> partial [4096] -> ReduceScatter
                                                         [collective 2]
      l2_W2 COLUMN-sharded by the RS output chunk -> partial [1024] out
      host sums the 4 partial outputs per branch (+ b2).
  - A prelude AllGather (registered via the bir_kernel_barrier sem
    mechanism, doorbell at ~7us) absorbs the runtime's global entry
    barrier (~20us after last-core arrival) + first-collective stream
    warmup (~15us) under the weight DMA.  Launch skew across cores is
    up to ~70us; early cores wait there instead of inside RS1.
  - PE program order matches DMA arrival order (l1, gih, then gh during
    the RS1 wait) so RS1's trigger — the group-critical event — is never
    queued behind whh's later arrival.  DMA order: wl1, wih, whh, w1, w2.
  - All weights are SBUF-resident: ~16.8MB per core streamed at
    ~330-380GB/s on the SP HWDGE ring; inputs + RS staging ride the ACT
    ring.
  - GEMVs run up to 4 concurrent PE column-group chains (tile_position
    col offsets 0/32/64/96), one m-tile per chain, K-accumulated in
    PSUM; W2 uses tw=256/pb=4 for 4 chains on a 1024-wide output.
  - Elementwise runs partition-major [128,4] per gate; RS outputs land
    as [12,128]/[8,128] and are transposed by a single identity matmul.
  - Keep-warm junk matmuls cover the RS2 wait so the PE HAM throttle
    (1.2GHz cold) doesn't re-engage before the W2 tail.
"""

import os

import numpy as np

X = 32
Y = 32
H1 = 5120
HID = 2048
H2 = 4096
OUT = 1024
IN = 1120
INP = 1152          # 9*128, slot 1120 = bias

NCORES = 8
TP = 4

M_L1 = H1 // TP     # 1280
M_G = 3 * (HID // TP)   # 1536 (r|z|n x 512)
HSH = HID // TP     # 512
Y3C = H2 // TP      # 1024 (ReduceScatter output chunk)

K_L1 = INP // 128   # 9
K_IH = H1 // 128    # 40
K_HH = HID // 128   # 16
K_W1 = HSH // 128   # 4
K_W2 = Y3C // 128   # 8

PL1 = 1536          # padded AllGather width (48*32, xbar-transposable)

# partition-major consts [128, CPM_TOT] f32: each [512] vec as [128,4]
CPM_BRZ_R = 0       # bih+bhh, r gate
CPM_BRZ_Z = 4
CPM_BHHN = 8
CPM_BIHN = 12
CPM_HSH = 16
CPM_B1 = 20         # l2_b1 RS chunk as [128, 8]
CPM_TOT = 28

FP8 = os.environ.get("KERNEL_FP8", "1") == "1"
FMAX = 12.0         # target max for e3m4 scaling (format max 15.5)

_CACHE = {}


def _build_nc(s_ih, s_w1, s_w2):
    import concourse.bass as bass  # noqa: F401
    import concourse.mybir as mybir
    import concourse.tile as tile
    from concourse import bacc

    f32 = mybir.dt.float32
    f16 = mybir.dt.float16
    wq = mybir.dt.float8e3 if FP8 else mybir.dt.float16

    nc = bacc.Bacc("TRN2", target_bir_lowering=False, debug=False,
                   num_devices=NCORES)

    idm_d = nc.dram_tensor("idm", [12, 12], f16, kind="ExternalInput")
    x0_d = nc.dram_tensor("x0", [128, K_L1], f16, kind="ExternalInput")
    hx_d = nc.dram_tensor("hx", [128, K_HH], f16, kind="ExternalInput")
    cpm_d = nc.dram_tensor("cpm", [128, CPM_TOT], f32, kind="ExternalInput")
    wl1_d = nc.dram_tensor("wl1", [3, 128, 3 * M_L1], wq, kind="ExternalInput")
    wih_d = nc.dram_tensor("wih", [4, 128, 10 * M_G], wq,
                           kind="ExternalInput")
    whh_d = nc.dram_tensor("whh", [1, 128, K_HH * M_G], wq,
                           kind="ExternalInput")
    w1_d = nc.dram_tensor("w1", [2, 128, K_W1 * H2 // 2], wq,
                          kind="ExternalInput")
    w2_d = nc.dram_tensor("w2", [1, 128, K_W2 * OUT], f16,
                          kind="ExternalInput")
    out_d = nc.dram_tensor("out", [4, 256], f32, kind="ExternalOutput")

    AF = mybir.ActivationFunctionType
    ALU = mybir.AluOpType
    groups = [[0, 1, 2, 3], [4, 5, 6, 7]]

    with tile.TileContext(nc) as tc, \
         tc.tile_pool(name="act", bufs=1) as apool, \
         tc.tile_pool(name="ps", bufs=8, space="PSUM") as ppool, \
         tc.tile_pool(name="dram", bufs=1, space="DRAM") as dpool:
        wpool = apool  # one SBUF pool: fewer TileContext exit barriers

        # ---- prelude-AllGather warm-up: registering these replica groups
        # makes the framework insert a tiny AllGather right after the
        # gpsimd preamble (doorbell at ~7us).  It absorbs the runtime's
        # global 8-core entry barrier (fires at the first collective,
        # ~15-20us AFTER the last core arrives — launch skew is up to
        # ~50us) plus the first-collective stream warmup, overlapped with
        # the weight DMA stream.  Unlike a user dummy collective_compute,
        # it completes ~8us earlier on the stream, so RS1 isn't pushed out.
        # Registered directly (not via bir_kernel_barrier_wait) because the
        # Tile scheduler's sim can't see the finalize-time AG increment and
        # would deadlock on the wait_ge; the AG instruction itself already
        # blocks gpsimd until the entry sync completes, which is all the
        # ordering RS1 needs.
        assert nc._bir_kernel_barrier_sem is not None
        nc._bir_kernel_barrier_sem_replica_groups.extend(
            set(g) for g in groups)

        # ---- input / consts DMAs (ACT HWDGE ring: lower fixed cost than
        # SWDGE and keeps GpSimd free to run only the collective stream)
        cpm = apool.tile([128, CPM_TOT], f32, tag="cpm", name="cpm_sb")
        nc.scalar.dma_start(cpm, cpm_d.ap())
        x0 = apool.tile([128, K_L1], f16, tag="x0", name="x0")
        nc.scalar.dma_start(x0, x0_d.ap())
        hx = apool.tile([128, K_HH], f16, tag="hx", name="hx")
        nc.scalar.dma_start(hx, hx_d.ap())
        idm = apool.tile([12, 12], f16, tag="idm", name="idm")
        nc.scalar.dma_start(idm, idm_d.ap())

        # ---- resident weight buffers
        wl1 = wpool.tile([128, K_L1 * M_L1], wq, tag="wl1", name="wl1_sb")
        wih = wpool.tile([128, K_IH * M_G], wq, tag="wih", name="wih_sb")
        whh = wpool.tile([128, K_HH * M_G], wq, tag="whh", name="whh_sb")
        w1 = wpool.tile([128, K_W1 * H2], wq, tag="w1", name="w1_sb")
        w2 = wpool.tile([128, K_W2 * OUT], f16, tag="w2", name="w2_sb")
        # wl1 heads the SP-ring queue so l1 can start before the bulk
        # stream saturates HBM (both HWDGE rings share the SDMA engines,
        # so a separate ring gives no priority).
        for p in range(3):
            nc.sync.dma_start(wl1[:, p * 3 * M_L1:(p + 1) * 3 * M_L1],
                              wl1_d.ap()[p])
        # wih directly after wl1: the l1->gih->RS1-trigger chain is the
        # group-critical path (RS1 completes only after the LAST core's
        # trigger); whh is only needed for gh, which hides under RS1.
        for p in range(4):
            nc.sync.dma_start(wih[:, p * 10 * M_G:(p + 1) * 10 * M_G],
                              wih_d.ap()[p])
        nc.sync.dma_start(whh[:, :], whh_d.ap()[0])
        for p in range(2):
            nc.sync.dma_start(w1[:, p * 2 * H2:(p + 1) * 2 * H2],
                              w1_d.ap()[p])
        nc.sync.dma_start(w2[:, :], w2_d.ap()[0])

        # ---- ACT LUT warmup
        warm = apool.tile([1, 32], f32, tag="warm", name="warm")
        nc.vector.memset(warm, 0.0)
        nc.scalar.activation(warm, warm, AF.Sigmoid)
        nc.scalar.activation(warm, warm, AF.Tanh)
        nc.scalar.activation(warm, warm, AF.Relu)

        ones = apool.tile([128, 1], f16, tag="ones", name="ones")
        nc.vector.memset(ones, 1.0)

        # ---- PSUM: 8 banks, allocated up-front.  The [65,512]-style
        # batched psum->SBUF copies read quadrant-gap rows, so zero those
        # regions early (off the critical path).
        l1p = ppool.tile([128, 512], f32, tag="l1p", bufs=1, name="l1p")
        ghp = ppool.tile([128, 512], f32, tag="ghp", bufs=1, name="ghp")
        gip = ppool.tile([128, 512], f32, tag="gip", bufs=1, name="gip")
        w1ps = [ppool.tile([128, 512], f32, tag=f"w1p{b}", bufs=1,
                           name=f"w1p{b}") for b in range(2)]
        w2p = l1p  # l1p's bank is free after y1; reuse for the W2 output
        smalls = ppool.tile([128, 42], f32, tag="smalls", bufs=1,
                            name="smalls")
        ghtp = smalls[:, 0:12]
        gtp = smalls[:, 12:24]
        x3ps = smalls[:, 24:32]
        xlps = smalls[:, 32:42]
        nc.vector.memset(l1p[0:65, :], 0.0)
        nc.vector.memset(gip[0:97, :], 0.0)
        nc.vector.memset(w1ps[0][0:97, :], 0.0)
        nc.vector.memset(w1ps[1][0:97, :], 0.0)

        def gemv(x_sb, w_sb, K, M, acc, xmap=None, tw=512, pb=3):
            """acc[j//pb][32*(j%pb), :mw] = W @ x for m-tile j (tw wide).
            pb concurrent column-group chains, K-accumulated in psum.
            k-major: one LDW per chain per k, all m-tiles streamed."""
            nm = (M + tw - 1) // tw
            mts = [(i * tw, min(tw, M - i * tw)) for i in range(nm)]

            def mm(j, k):
                kk = xmap(k) if xmap else k
                m0, mw = mts[j]
                c = 32 * (j % pb)
                nc.tensor.matmul(
                    acc[j // pb][c:c + 1, :mw],
                    x_sb[:, kk:kk + 1],
                    w_sb[:, k * M + m0: k * M + m0 + mw],
                    start=(k == 0), stop=(k == K - 1),
                    tile_position=(0, c),
                )

            # k-major for the bulk; the tail chunks run m-tile-major so
            # early m-tiles retire first and their psum->SBUF copies
            # overlap the remaining chains.  Small-K gemvs (W1) go fully
            # m-tile-major so the first bank's staging overlaps the rest.
            tail = K if K <= 4 else (min(6, K) if K > 6 else 0)
            for k in range(K - tail):
                for j in range(nm):
                    mm(j, k)
            for j in range(nm):
                for k in range(K - tail, K):
                    mm(j, k)
            return mts

        def to_part(src, col0, cols, ps, pcol0):
            """src[0, col0+128*c : col0+128*(c+1)] (fp16 SBUF, partition 0)
            -> ps[:, pcol0+c] for c in range(cols), via K=1 matmuls."""
            for c in range(cols):
                s0 = col0 + c * 128
                nc.tensor.matmul(ps[:, pcol0 + c:pcol0 + c + 1],
                                 src[0:1, s0:s0 + 128],
                                 ones[0:1, 0:1],
                                 start=True, stop=True)

        # ---- l1: relu(W@x)*s_ih, row-sharded (bias folded in weights)
        gemv(x0, wl1, K_L1, M_L1, [l1p])
        # ---- l1 output stays local: one [65,512] relu keeps all lanes
        # busy (vs 3 serial single-lane [1,mw] ops), then on-PE transpose
        # to x1loc [128,10] for the column-sharded gi.
        ytmp = apool.tile([65, 512], f16, tag="ytmp", name="ytmp")
        nc.scalar.activation(ytmp, l1p[0:65, :], AF.Relu, scale=float(s_ih))
        for c in range(10):
            r, q = 32 * (c // 4), (c % 4) * 128
            nc.tensor.matmul(xlps[:, c:c + 1],
                             ytmp[r:r + 1, q:q + 128],
                             ones[r:r + 1, 0:1],
                             start=True, stop=True)
        x1loc = apool.tile([128, 10], f16, tag="x1loc", name="x1loc")
        nc.scalar.copy(x1loc, xlps)

        # gih BEFORE the gh gemv in PE order: wih is streamed right after
        # wl1 (whh after), and the PE executes its queue in order — with
        # gh first, gih's tail (and so RS1's trigger, the group-critical
        # event) would be queued behind whh's later DMA arrival.
        gacc = [gip, w1ps[0], w1ps[1]]
        gemv(x1loc, wih, 10, 3 * HID, gacc, tw=512, pb=4)
        rsin1 = dpool.tile([12, 512], f16, tag="rsin1", name="rsin1")
        rs1out = dpool.tile([12, 128], f16, tag="rs1out", name="rs1out")
        for b in range(3):
            yg = apool.tile([97, 512], f16, tag=f"yg{b}", name=f"yg{b}")
            nc.scalar.copy(yg, gacc[b][0:97, :])
            nc.scalar.dma_start(rsin1[4 * b:4 * b + 4, :], yg[0:97:32, :])
        nc.gpsimd.collective_compute(
            "ReduceScatter", ALU.add, replica_groups=groups,
            ins=[rsin1.opt()], outs=[rs1out.opt()])

        # ---- gru: gh = Whh @ (hn*s_hh); runs during the RS1 wait
        gemv(hx, whh, K_HH, M_G, [ghp], tw=384, pb=4)
        ghs = apool.tile([1, M_G], f16, tag="ghs", name="ghs")
        for g in range(4):
            nc.scalar.copy(ghs[:, 384 * g:384 * (g + 1)],
                           ghp[32 * g:32 * g + 1, :384])
        to_part(ghs, 0, 12, ghtp, 0)
        # ghb = gh + [brz_r | brz_z | bhhn]  (partition-major [128,12])
        ghb = apool.tile([128, 12], f32, tag="ghb", name="ghb")
        nc.vector.tensor_add(ghb, ghtp, cpm[:, 0:12])

        # RS1 result lands as [12,128] in SBUF; one identity matmul
        # transposes it to partition-major [128,12] (vs 12 serial K=1
        # matmuls) — this sits on the group-critical post-RS1 path.
        gis = apool.tile([12, 128], f16, tag="gis", name="gis")
        nc.scalar.dma_start(gis, rs1out)
        nc.tensor.matmul(gtp, gis, idm, start=True, stop=True)

        # ---- gru cell elementwise, partition-major [128, 4] per gate
        rz = apool.tile([128, 8], f32, tag="rz", name="rz")
        nc.vector.tensor_add(rz, gtp[:, 0:8], ghb[:, 0:8])
        nc.scalar.activation(rz, rz, AF.Sigmoid)
        tn = apool.tile([128, 4], f32, tag="tn", name="tn")
        nc.vector.tensor_mul(tn, rz[:, 0:4], ghb[:, 8:12])      # r*(ghn+bhhn)
        tn2 = apool.tile([128, 4], f32, tag="tn2", name="tn2")
        nc.vector.tensor_add(tn2, gtp[:, 8:12], cpm[:, CPM_BIHN:CPM_BIHN + 4])
        nc.vector.tensor_add(tn, tn, tn2)
        nc.scalar.activation(tn, tn, AF.Tanh)                   # n
        # s_w1 (the fp8 scale of l2_W1) is folded into h' here for free:
        # cpm's HSH column holds h*s_w1 (host-prepped), so
        # td = s*h - s*n, x2 = s*n + z*(s*h - s*n) = s_w1 * h'.
        td = apool.tile([128, 4], f32, tag="td", name="td")
        nc.vector.scalar_tensor_tensor(td, tn, -float(s_w1),
                                       cpm[:, CPM_HSH:CPM_HSH + 4],
                                       ALU.mult, ALU.add)       # s*(h-n)
        nc.vector.tensor_mul(td, rz[:, 4:8], td)                # z*s*(h-n)
        x2 = apool.tile([128, 4], f16, tag="x2", name="x2")
        nc.vector.scalar_tensor_tensor(x2, tn, float(s_w1), td,
                                       ALU.mult, ALU.add)       # s_w1*h'

        # ---- l2_W1 column-sharded: partial[4096] = W1[:, shard] @ h'
        gemv(x2, w1, K_W1, H2, w1ps, pb=4)
        rsin = dpool.tile([8, 512], f16, tag="rsin", name="rsin")
        rsout = dpool.tile([8, 128], f16, tag="rsout", name="rsout")
        for b in range(2):
            yb = apool.tile([97, 512], f16, tag=f"yb{b}", name=f"yb{b}")
            nc.scalar.copy(yb, w1ps[b][0:97, :])
            nc.scalar.dma_start(rsin[4 * b:4 * b + 4, :], yb[0:97:32, :])

        # keep-warm junk matmuls: span the RS2 wait (staging + ~6.5us RS +
        # y3p DMA ~= 12us) so the PE HAM throttle doesn't re-engage before
        # the y3/W2 tail.  Slightly undersized: overshoot delays the y3
        # transpose directly, undershoot only risks a 2x-slower W2.
        for i in range(30):
            nc.tensor.matmul(ghp[0:1, :384], x2[:, 0:1], w2[:, 0:384],
                             start=True, stop=True, tile_position=(0, 0))
        nc.gpsimd.collective_compute(
            "ReduceScatter", ALU.add, replica_groups=groups,
            ins=[rsin.opt()], outs=[rsout.opt()])
        y3p = apool.tile([8, 128], f16, tag="y3p", name="y3p")
        nc.scalar.dma_start(y3p, rsout)

        # transpose to [128, 8] with one identity matmul, then bias+relu
        nc.tensor.matmul(x3ps, y3p, idm[0:8, 0:8], start=True, stop=True)
        x3t = apool.tile([128, 8], f32, tag="x3t", name="x3t")
        nc.vector.tensor_add(x3t, x3ps, cpm[:, CPM_B1:CPM_B1 + 8])
        x3 = apool.tile([128, 8], f16, tag="x3", name="x3")
        # relu(x*s) = s*relu(x) for s>0: folds the fp8 scale of l2_W2 in.
        nc.scalar.activation(x3, x3t, AF.Relu, scale=float(s_w2))

        # ---- l2_W2 column-sharded: partial [1024] out.  tw=256/pb=4 runs
        # 4 concurrent PE column-group chains (2x the 2-chain tw=512 rate).
        gemv(x3, w2, K_W2, OUT, [w2p], tw=256, pb=4)
        yo = apool.tile([97, 256], f32, tag="yo", name="yo")
        nc.scalar.copy(yo, w2p[0:97, :256])
        nc.scalar.dma_start(out_d.ap(), yo[0:97:32, :])

    nc.finalize()
    return nc


def _pow2_scale(*arrs):
    m = max(float(np.abs(a).max()) for a in arrs)
    return float(2.0 ** np.ceil(np.log2(max(m, 1e-30) / FMAX)))


def _qpack(wt, K, M, npieces, npw):
    """[K*128, M] input-major transposed weight -> [npieces, 128, K*M/np]
    chunk-major packed (element [p, k*M+m] = wt[k*128+p, m])."""
    v = (wt.reshape(K, 128, M).transpose(1, 0, 2)
         .reshape(128, npieces, K * M // npieces).transpose(1, 0, 2))
    return np.ascontiguousarray(v).astype(npw)


def _pm(vec):
    """[n*128] -> [128, n] partition-major (element u -> [u%128, u//128])."""
    return np.ascontiguousarray(np.asarray(vec, np.float32)
                                .reshape(-1, 128).T)


def _prep_core(r, xvec, hn, l1W, l1b, Wih, Whh, bih, bhh, W1, b1, W2, b2,
               s_l1, s_ih, s_hh, s_w1, s_w2, npw):
    f32 = np.float32
    f16 = np.float16

    rs = slice(r * M_L1, (r + 1) * M_L1)
    wt = np.zeros((INP, M_L1), f32)
    wt[:IN] = l1W[rs].T
    wt[IN] = l1b[rs]
    wl1 = _qpack(wt / s_l1, K_L1, M_L1, 3, npw)

    gsl = [slice(g * HID + r * HSH, g * HID + (r + 1) * HSH) for g in range(3)]
    gidx = np.concatenate([np.arange(s.start, s.stop) for s in gsl])
    # full rank-major gate permutation (rank r' block = r|z|n of shard r')
    pfull = np.concatenate([
        np.arange(g * HID + rr * HSH, g * HID + (rr + 1) * HSH)
        for rr in range(TP) for g in range(3)])
    wih = _qpack(np.ascontiguousarray(
        Wih[pfull][:, r * M_L1:(r + 1) * M_L1].T) / s_ih, 10, 3 * HID,
        4, npw)
    whh = _qpack(np.ascontiguousarray(Whh[gidx].T) / s_hh, K_HH, M_G, 1, npw)

    w1 = _qpack(np.ascontiguousarray(W1[:, r * HSH:(r + 1) * HSH].T) / s_w1,
                K_W1, H2, 2, npw)
    w2 = _qpack(np.ascontiguousarray(W2[:, r * Y3C:(r + 1) * Y3C].T) / s_w2,
                K_W2, OUT, 1, f16)

    bsum = bih + bhh
    cpm = np.concatenate([
        _pm(bsum[gsl[0]]), _pm(bsum[gsl[1]]),      # brz_r, brz_z
        _pm(bhh[gsl[2]]), _pm(bih[gsl[2]]),        # bhhn, bihn
        _pm(hn[r * HSH:(r + 1) * HSH]) * s_w1,     # h shard * s_w1
        _pm(b1[r * Y3C:(r + 1) * Y3C]),            # RS chunk bias
    ], axis=1).astype(f32)
    assert cpm.shape == (128, CPM_TOT)

    x = np.zeros(INP, f32)
    x[:IN] = xvec
    x[IN] = 1.0
    x0 = np.ascontiguousarray((x * s_l1).reshape(K_L1, 128).T).astype(f16)
    hx = np.ascontiguousarray((hn * s_hh).reshape(K_HH, 128).T).astype(f16)

    return {
        "x0": x0, "hx": hx, "cpm": cpm, "idm": np.eye(12, dtype=f16),
        "wl1": wl1, "wih": wih, "whh": whh, "w1": w1, "w2": w2,
    }


LAST_RESULT = None


def kernel(state_inno, observation_inno, diff_state, diff_obs,
           linearization_error, Jacobian,
           l1_W, l1_b, gru1_Wih, gru1_Whh, gru1_bih, gru1_bhh,
           l2_W1, l2_b1, l2_W2, l2_b2,
           l3_W, l3_b, gru2_Wih, gru2_Whh, gru2_bih, gru2_bhh,
           l4_W1, l4_b1, l4_W2, l4_b2, hn1, hn2):
    global LAST_RESULT
    from concourse.bass_utils import run_bass_kernel_spmd
    import concourse.mybir as mybir

    npw = mybir.dt.np(mybir.dt.float8e3) if FP8 else np.float16

    a = lambda v: np.asarray(v, dtype=np.float32)
    input1 = np.concatenate([a(state_inno), a(diff_state),
                             a(linearization_error), a(Jacobian)]).reshape(-1)
    input2 = np.concatenate([a(observation_inno), a(diff_obs),
                             a(linearization_error), a(Jacobian)]).reshape(-1)

    branches = [
        (input1, a(hn1).reshape(-1), a(l1_W), a(l1_b).reshape(-1),
         a(gru1_Wih), a(gru1_Whh), a(gru1_bih).reshape(-1),
         a(gru1_bhh).reshape(-1), a(l2_W1), a(l2_b1).reshape(-1),
         a(l2_W2), a(l2_b2).reshape(-1)),
        (input2, a(hn2).reshape(-1), a(l3_W), a(l3_b).reshape(-1),
         a(gru2_Wih), a(gru2_Whh), a(gru2_bih).reshape(-1),
         a(gru2_bhh).reshape(-1), a(l4_W1), a(l4_b1).reshape(-1),
         a(l4_W2), a(l4_b2).reshape(-1)),
    ]

    if FP8:
        s_l1 = _pow2_scale(
            np.concatenate([branches[0][2].ravel(), branches[0][3]]),
            np.concatenate([branches[1][2].ravel(), branches[1][3]]))
        s_ih = _pow2_scale(branches[0][4], branches[1][4])
        s_hh = _pow2_scale(branches[0][5], branches[1][5])
        s_w1 = _pow2_scale(branches[0][8], branches[1][8])
    else:
        s_l1 = s_ih = s_hh = s_w1 = 1.0
    # l2_W2 stays f16 (fp8 on BOTH l2 weights pushed rel err past the
    # 2e-2 budget; W2's quant error hits the output unaveraged).  W1 in
    # fp8 halves the critical-path W1 gemv's xbus time.
    s_w2 = 1.0

    if "nc" not in _CACHE:
        _CACHE["nc"] = (_build_nc(s_ih, s_w1, s_w2),
                        s_l1, s_ih, s_hh, s_w1, s_w2)
    nc, s_l1, s_ih, s_hh, s_w1, s_w2 = _CACHE["nc"]

    in_maps = [_prep_core(c % TP, *branches[c // TP],
                          s_l1, s_ih, s_hh, s_w1, s_w2, npw) for c in range(NCORES)]

    kwargs = {}
    if os.environ.get("KERNEL_TRACE"):
        cores = os.environ.get("KERNEL_TRACE_CORES", "0")
        kwargs.update(trace=True,
                      trace_cores=[int(c) for c in cores.split(",")])

    res = run_bass_kernel_spmd(nc, in_maps, core_ids=list(range(NCORES)),
                               **kwargs)
    LAST_RESULT = res
    outs = [res.results[c]["out"].reshape(-1) for c in range(NCORES)]
    b2P = branches[0][11]
    b2S = branches[1][11]
    Pk = (sum(outs[:TP]) + b2P).reshape(X, X).astype(np.float32)
    Sk = (sum(outs[TP:]) + b2S).reshape(Y, Y).astype(np.float32)
    return Pk, Sk

